# revision 10
# baseline (speedup 1.0000x reference)
"""CrossAttention TRN2 kernel — context-parallel over (batch, seq-chunk), all-bf16, no collectives.

8 cores: core c -> batch b=c//4, seq chunk j=c%4 (512 query rows).
v4: chunked contiguous casting loads (gpsimd swdge, consumption order),
PE transposes (identity matmul) for ctxT/xT, proj PSUM->SBUF copies on ACT,
softmax reciprocal via DVE reciprocal_approx_fast off an SBUF copy of the
denominator, output bias via DVE add, attention pipelined in 2-chunk PSUM
groups double-buffered 3-deep against ACT exp.

Per core (all matmul inputs bf16, PSUM accumulation fp32):
  load:  x chunk + full ctx_b + W* cast to bf16 via gpsimd [128,*] block DMAs
  tpose: ctxT[p,kc,m], xT[p,kc,n] via PE transpose + DVE copy out of PSUM
  proj:  qT = Wq.T@xT; kT = Wk.T@ctxT; vaug = ctx@Wv (ones-augmented)
  attn:  S.T[m,n] = kT_h.T @ qT_h; exp on ACT -> bf16 SBUF;
         AV: oX[65,n] += vaug_h.T @ expST (row 64 = denom);
         normalize: ACT den copy + DVE recip_approx + gpsimd bcast + DVE mul
  out:   out[n,1024] = oT.T @ Wo (+bias via DVE add) -> fp32 DMA out
PSUM: proj: pst 2 + pp 4 banks; attn: S 3x2 + oX 2; out: 2x1 banks.
"""
import sys
sys.path.insert(0, '/opt/trn_rl_repo')
import contextlib
import numpy as np
import concourse.bass as bass
import concourse.mybir as mybir
import concourse.tile as tile
from concourse import bacc

F32 = mybir.dt.float32
BF16 = mybir.dt.bfloat16
AF = mybir.ActivationFunctionType

B, N, M, KDIM, H, D = 2, 2048, 2048, 1024, 8, 64
INNER = H * D          # 512
NC = 512               # query rows per core chunk
SCALE = D ** -0.5      # 0.125
KC = KDIM // 128       # 8 k-chunks
DC = INNER // 128      # 4 inner chunks (= head pairs)
NT = NC // 128         # 4 n-tiles per core
MC = M // 128          # 16 m-chunks
VW = 2 * (D + 1)       # 130: [vA(64) | 1 | vB(64) | 1] per head pair
GRP = 2                # m-chunks per exp group


def build_kernel():
    nc = bacc.Bacc("TRN2", target_bir_lowering=False, debug=False, num_devices=8)
    X = nc.dram_tensor("xc", [NC, KDIM], F32, kind="ExternalInput")
    CTX = nc.dram_tensor("ctxc", [M, KDIM], F32, kind="ExternalInput")
    WQ = nc.dram_tensor("Wq", [KDIM, INNER], F32, kind="ExternalInput")
    WK = nc.dram_tensor("Wk", [KDIM, INNER], F32, kind="ExternalInput")
    WV = nc.dram_tensor("Wv", [KDIM, INNER], F32, kind="ExternalInput")
    WO = nc.dram_tensor("Wo", [INNER, KDIM], F32, kind="ExternalInput")
    BO = nc.dram_tensor("bo", [1, KDIM], F32, kind="ExternalInput")
    OUT = nc.dram_tensor("outc", [NC, KDIM], F32, kind="ExternalOutput")

    with tile.TileContext(nc) as tc:
        with contextlib.ExitStack() as ctx:
            sb = ctx.enter_context(tc.tile_pool(name="sb", bufs=1))
            stage = ctx.enter_context(tc.tile_pool(name="stage", bufs=3))

            # ---------- persistent SBUF tiles ----------
            xn = sb.tile([128, NT * KDIM], BF16, tag="xn")
            xn3 = xn[:].rearrange("p (t k) -> p t k", t=NT)
            ctxn = [sb.tile([128, 4 * KDIM], BF16, tag=f"ctxn{g}", name=f"ctxn{g}")
                    for g in range(4)]
            wq = sb.tile([128, KC * INNER], BF16, tag="wq")
            wk = sb.tile([128, KC * INNER], BF16, tag="wk")
            wv = sb.tile([128, KC * INNER], BF16, tag="wv")
            wo = sb.tile([128, DC * KDIM], BF16, tag="wo")
            wq3 = wq[:].rearrange("p (k d) -> p k d", k=KC)
            wk3 = wk[:].rearrange("p (k d) -> p k d", k=KC)
            wv3 = wv[:].rearrange("p (k d) -> p k d", k=KC)
            wo3 = wo[:].rearrange("p (i o) -> p i o", i=DC)
            ctxT = sb.tile([128, KC * M], BF16, tag="ctxT")
            ctxT3 = ctxT[:].rearrange("p (k m) -> p k m", k=KC)
            xT = sb.tile([128, KC * NC], BF16, tag="xT")
            xT3 = xT[:].rearrange("p (k n) -> p k n", k=KC)
            kT = [sb.tile([128, M], BF16, tag=f"kT{dc}", name=f"kT{dc}") for dc in range(DC)]
            vaug = [sb.tile([128, VW * DC], BF16, tag=f"vg{mt}", name=f"vg{mt}")
                    for mt in range(MC)]
            qT = [sb.tile([128, NC], BF16, tag=f"qT{dc}", name=f"qT{dc}") for dc in range(DC)]
            oT = [sb.tile([128, NC], BF16, tag=f"oT{hp}", name=f"oT{hp}") for hp in range(DC)]
            bo_r = sb.tile([1, KDIM], F32, tag="bo_r")
            bias_bc = sb.tile([128, KDIM], F32, tag="bias_bc")

            # ---------- input DMAs (gpsimd casting swdge), contiguous blocks,
            # consumption order ----------
            def load_w(w3, WD, nchunk):
                for k in range(nchunk):
                    nc.gpsimd.dma_start(w3[:, k, :], WD[128 * k:128 * (k + 1), :])

            def load_ctx(g):
                cg = ctxn[g][:].rearrange("p (t k) -> p t k", t=4)
                for t in range(4):
                    r0 = 512 * g + 128 * t
                    nc.gpsimd.dma_start(cg[:, t, :], CTX[r0:r0 + 128, :])

            for t in range(NT):
                nc.gpsimd.dma_start(xn3[:, t, :], X[128 * t:128 * (t + 1), :])
            load_w(wq3, WQ, KC)
            load_w(wk3, WK, KC)
            load_ctx(0)
            load_w(wv3, WV, KC)
            load_ctx(1)
            load_ctx(2)
            load_ctx(3)
            load_w(wo3, WO, DC)
            nc.sync.dma_start(bo_r[:], BO[:])
            nc.gpsimd.partition_broadcast(bias_bc[:], bo_r[:])

            # ones columns of vaug (constant, written once)
            for mt in range(MC):
                ones = vaug[mt][:].rearrange("p (hp w) -> p hp w", hp=DC).rearrange(
                    "p hp (two dd) -> p hp two dd", two=2)[:, :, :, D:D + 1]
                nc.vector.memset(ones, 1.0)

            # ---------- xbar transposes (scalar hwdge, arrival order) + projections ----------
            with tc.tile_pool(name="pp", bufs=4, space="PSUM") as pp:
                for t in range(NT):
                    nc.scalar.dma_start_transpose(
                        xT3[:, :, 128 * t:128 * (t + 1)], xn3[:, t, :])
                for dc in range(DC):
                    p = pp.tile([128, NC], F32, tag="pp")
                    for k in range(KC):
                        nc.tensor.matmul(p[:], wq3[:, k, 128 * dc:128 * (dc + 1)],
                                         xT3[:, k, :], start=(k == 0), stop=(k == KC - 1))
                    nc.scalar.copy(qT[dc][:], p[:])

                for g in range(4):
                    cg = ctxn[g][:].rearrange("p (t k) -> p t k", t=4)
                    for t in range(4):
                        nc.scalar.dma_start_transpose(
                            ctxT3[:, :, 128 * (4 * g + t):128 * (4 * g + t + 1)],
                            cg[:, t, :])
                    for dc in range(DC):
                        p = pp.tile([128, 512], F32, tag="pp")
                        for k in range(KC):
                            nc.tensor.matmul(p[:], wk3[:, k, 128 * dc:128 * (dc + 1)],
                                             ctxT3[:, k, 512 * g:512 * (g + 1)],
                                             start=(k == 0), stop=(k == KC - 1))
                        nc.scalar.copy(kT[dc][:, 512 * g:512 * (g + 1)], p[:])
                    for t in range(4):
                        mt = 4 * g + t
                        p = pp.tile([128, 512], F32, tag="pp")
                        for k in range(KC):
                            nc.tensor.matmul(p[:], ctxT3[:, k, 128 * mt:128 * (mt + 1)],
                                             wv3[:, k, :], start=(k == 0), stop=(k == KC - 1))
                        pv = p[:].rearrange("p (hp two d) -> p hp two d", hp=DC, two=2)
                        tv = vaug[mt][:].rearrange("p (hp w) -> p hp w", hp=DC)[
                            :, :, 0:VW].rearrange(
                            "p hp (two dd) -> p hp two dd", two=2)[:, :, :, 0:D]
                        nc.vector.tensor_copy(tv, pv)

            # ---------- attention ----------
            NG = MC // GRP  # 8 groups of 2 m-chunks
            with (tc.tile_pool(name="psS", bufs=3, space="PSUM") as psS,
                  tc.tile_pool(name="psO", bufs=2, space="PSUM") as psO,
                  tc.tile_pool(name="se", bufs=3) as se):
                for hp in range(DC):
                    for head in range(2):
                        bk = 64 * head
                        vb = VW * hp + (D + 1) * head
                        oX = psO.tile([128, NC], F32, tag="oX")

                        def do_av(pend):
                            g_p, e_p = pend
                            for i in range(GRP):
                                mc = GRP * g_p + i
                                v = vaug[mc][:, vb:vb + D + 1]
                                nc.tensor.matmul(oX[0:D + 1, :], v,
                                                 e_p[:, 512 * i:512 * (i + 1)],
                                                 start=(mc == 0), stop=(mc == MC - 1))

                        pending = None
                        for g in range(NG):
                            s = psS.tile([128, 512 * GRP], F32, tag="s")
                            for i in range(GRP):
                                mc = GRP * g + i
                                ksl = kT[hp][bk:bk + 64, 128 * mc:128 * (mc + 1)]
                                nc.tensor.matmul(s[:, 512 * i:512 * (i + 1)], ksl,
                                                 qT[hp][bk:bk + 64, :],
                                                 start=True, stop=True)
                            if pending is not None:
                                do_av(pending)
                            e = se.tile([128, 512 * GRP], BF16, tag="e")
                            nc.scalar.activation(e[:], s[:], AF.Exp, bias=0.0, scale=SCALE)
                            pending = (g, e)
                        do_av(pending)
                        den = stage.tile([1, NC], F32, tag="den")
                        nc.scalar.copy(den[:], oX[D:D + 1, :])
                        rec = stage.tile([1, NC], F32, tag="rec")
                        nc.vector.reciprocal_approx_fast(rec[:], den[:])
                        rec_b = stage.tile([D, NC], F32, tag="rec_b")
                        nc.gpsimd.partition_broadcast(rec_b[:], rec[:])
                        nc.vector.tensor_mul(oT[hp][bk:bk + D, :], oX[0:D, :], rec_b[:])

            # ---------- O projection + bias ----------
            with (tc.tile_pool(name="psD", bufs=2, space="PSUM") as psD,
                  tc.tile_pool(name="so", bufs=2) as so):
                for nt in range(NT):
                    for hf in range(2):
                        p = psD.tile([128, 512], F32, tag="pout")
                        for ic in range(DC):
                            nc.tensor.matmul(p[:], oT[ic][:, 128 * nt:128 * (nt + 1)],
                                             wo3[:, ic, 512 * hf:512 * (hf + 1)],
                                             start=(ic == 0), stop=(ic == DC - 1))
                        osb = so.tile([128, 512], F32, tag="osb")
                        nc.vector.tensor_add(osb[:], p[:], bias_bc[:, 512 * hf:512 * (hf + 1)])
                        nc.sync.dma_start(
                            OUT[128 * nt:128 * (nt + 1), 512 * hf:512 * (hf + 1)], osb[:])
    nc.compile()
    return nc


def shard_inputs(inputs):
    """full inputs dict -> list of 8 per-core in_maps"""
    x, ctx = np.asarray(inputs["x"]), np.asarray(inputs["context"])
    maps = []
    for c in range(8):
        b, j = c // 4, c % 4
        maps.append({
            "xc": np.ascontiguousarray(x[b, NC * j:NC * (j + 1), :]),
            "ctxc": np.ascontiguousarray(ctx[b]),
            "Wq": np.asarray(inputs["Wq"]), "Wk": np.asarray(inputs["Wk"]),
            "Wv": np.asarray(inputs["Wv"]), "Wo": np.asarray(inputs["Wo"]),
            "bo": np.asarray(inputs["bo"]).reshape(1, KDIM),
        })
    return maps


def unshard_outputs(results):
    out = np.empty((B, N, KDIM), dtype=np.float32)
    for c in range(8):
        b, j = c // 4, c % 4
        out[b, NC * j:NC * (j + 1), :] = results[c]["outc"]
    return out


_CACHED = {}


def kernel(**inputs):
    """Full unsharded inputs -> full output [2, 2048, 1024] fp32. Runs on 8 NeuronCores."""
    from concourse.bass_utils import run_bass_kernel_spmd
    if "nc" not in _CACHED:
        _CACHED["nc"] = build_kernel()
    nc = _CACHED["nc"]
    maps = shard_inputs(inputs)
    res = run_bass_kernel_spmd(nc, maps, list(range(8)))
    return unshard_outputs(res.results)


# revision 11
# speedup vs baseline: 1.4687x; 1.4687x over previous
"""CrossAttention TRN2 kernel — context-parallel over (batch, seq-chunk), all-bf16, no collectives.

8 cores: core c -> batch b=c//4, seq chunk j=c%4 (512 query rows).
v4: chunked contiguous casting loads (gpsimd swdge, consumption order),
PE transposes (identity matmul) for ctxT/xT, proj PSUM->SBUF copies on ACT,
softmax reciprocal via DVE reciprocal_approx_fast off an SBUF copy of the
denominator, output bias via DVE add, attention pipelined in 2-chunk PSUM
groups double-buffered 3-deep against ACT exp.

Per core (all matmul inputs bf16, PSUM accumulation fp32):
  load:  x chunk + full ctx_b + W* cast to bf16 via gpsimd [128,*] block DMAs
  tpose: ctxT[p,kc,m], xT[p,kc,n] via PE transpose + DVE copy out of PSUM
  proj:  qT = Wq.T@xT; kT = Wk.T@ctxT; vaug = ctx@Wv (ones-augmented)
  attn:  S.T[m,n] = kT_h.T @ qT_h; exp on ACT -> bf16 SBUF;
         AV: oX[65,n] += vaug_h.T @ expST (row 64 = denom);
         normalize: ACT den copy + DVE recip_approx + gpsimd bcast + DVE mul
  out:   out[n,1024] = oT.T @ Wo (+bias via DVE add) -> fp32 DMA out
PSUM: proj: pst 2 + pp 4 banks; attn: S 3x2 + oX 2; out: 2x1 banks.
"""
import sys
sys.path.insert(0, '/opt/trn_rl_repo')
import contextlib
import numpy as np
import concourse.bass as bass
import concourse.mybir as mybir
import concourse.tile as tile
from concourse import bacc
from concourse.masks import make_identity

F32 = mybir.dt.float32
BF16 = mybir.dt.bfloat16
AF = mybir.ActivationFunctionType

B, N, M, KDIM, H, D = 2, 2048, 2048, 1024, 8, 64
INNER = H * D          # 512
NC = 512               # query rows per core chunk
SCALE = D ** -0.5      # 0.125
KC = KDIM // 128       # 8 k-chunks
DC = INNER // 128      # 4 inner chunks (= head pairs)
NT = NC // 128         # 4 n-tiles per core
MC = M // 128          # 16 m-chunks
VW = 2 * (D + 1)       # 130: [vA(64) | 1 | vB(64) | 1] per head pair
GRP = 2                # m-chunks per exp group


def build_kernel():
    nc = bacc.Bacc("TRN2", target_bir_lowering=False, debug=False, num_devices=8)
    X = nc.dram_tensor("xc", [NC, KDIM], F32, kind="ExternalInput")
    CTX = nc.dram_tensor("ctxc", [M, KDIM], F32, kind="ExternalInput")
    WQ = nc.dram_tensor("Wq", [KDIM, INNER], F32, kind="ExternalInput")
    WK = nc.dram_tensor("Wk", [KDIM, INNER], F32, kind="ExternalInput")
    WV = nc.dram_tensor("Wv", [KDIM, INNER], F32, kind="ExternalInput")
    WO = nc.dram_tensor("Wo", [INNER, KDIM], F32, kind="ExternalInput")
    BO = nc.dram_tensor("bo", [1, KDIM], F32, kind="ExternalInput")
    OUT = nc.dram_tensor("outc", [NC, KDIM], F32, kind="ExternalOutput")

    with tile.TileContext(nc) as tc:
        with contextlib.ExitStack() as ctx:
            sb = ctx.enter_context(tc.tile_pool(name="sb", bufs=1))
            stage = ctx.enter_context(tc.tile_pool(name="stage", bufs=3))

            ident = sb.tile([128, 128], BF16, tag="ident")
            make_identity(nc, ident[:])

            # ---------- persistent SBUF tiles ----------
            xn = sb.tile([128, NT * KDIM], BF16, tag="xn")
            xn3 = xn[:].rearrange("p (t k) -> p t k", t=NT)
            ctxn = [sb.tile([128, 4 * KDIM], BF16, tag=f"ctxn{g}", name=f"ctxn{g}")
                    for g in range(4)]
            wq = sb.tile([128, KC * INNER], BF16, tag="wq")
            wk = sb.tile([128, KC * INNER], BF16, tag="wk")
            wv = sb.tile([128, KC * INNER], BF16, tag="wv")
            wo = sb.tile([128, DC * KDIM], BF16, tag="wo")
            wq3 = wq[:].rearrange("p (k d) -> p k d", k=KC)
            wk3 = wk[:].rearrange("p (k d) -> p k d", k=KC)
            wv3 = wv[:].rearrange("p (k d) -> p k d", k=KC)
            wo3 = wo[:].rearrange("p (i o) -> p i o", i=DC)
            ctxT = sb.tile([128, KC * M], BF16, tag="ctxT")
            ctxT3 = ctxT[:].rearrange("p (k m) -> p k m", k=KC)
            xT = sb.tile([128, KC * NC], BF16, tag="xT")
            xT3 = xT[:].rearrange("p (k n) -> p k n", k=KC)
            kT = [sb.tile([128, M], BF16, tag=f"kT{dc}", name=f"kT{dc}") for dc in range(DC)]
            vaug = [sb.tile([128, VW * DC], BF16, tag=f"vg{mt}", name=f"vg{mt}")
                    for mt in range(MC)]
            qT = [sb.tile([128, NC], BF16, tag=f"qT{dc}", name=f"qT{dc}") for dc in range(DC)]
            oT = [sb.tile([128, NC], BF16, tag=f"oT{hp}", name=f"oT{hp}") for hp in range(DC)]
            bo_r = sb.tile([1, KDIM], F32, tag="bo_r")
            bias_bc = sb.tile([128, KDIM], F32, tag="bias_bc")

            # ---------- input DMAs (gpsimd casting swdge), contiguous blocks,
            # consumption order ----------
            def load_w(w3, WD, nchunk):
                for k in range(nchunk):
                    nc.gpsimd.dma_start(w3[:, k, :], WD[128 * k:128 * (k + 1), :])

            def load_ctx(g):
                cg = ctxn[g][:].rearrange("p (t k) -> p t k", t=4)
                for t in range(4):
                    r0 = 512 * g + 128 * t
                    nc.gpsimd.dma_start(cg[:, t, :], CTX[r0:r0 + 128, :])

            for t in range(NT):
                nc.gpsimd.dma_start(xn3[:, t, :], X[128 * t:128 * (t + 1), :])
            load_w(wq3, WQ, KC)
            load_w(wk3, WK, KC)
            load_ctx(0)
            load_w(wv3, WV, KC)
            load_ctx(1)
            load_ctx(2)
            load_ctx(3)
            load_w(wo3, WO, DC)
            nc.sync.dma_start(bo_r[:], BO[:])
            nc.gpsimd.partition_broadcast(bias_bc[:], bo_r[:])

            # ones columns of vaug (constant, written once)
            for mt in range(MC):
                ones = vaug[mt][:].rearrange("p (hp w) -> p hp w", hp=DC).rearrange(
                    "p hp (two dd) -> p hp two dd", two=2)[:, :, :, D:D + 1]
                nc.vector.memset(ones, 1.0)

            # ---------- PE transposes + projections ----------
            def transpose_tile(pst, dstT3, src2, t):
                # src2: [128, KDIM] bf16 rows; writes dstT3[:, :, 128t:128(t+1)]
                for kg in range(KC // 4):
                    p = pst.tile([128, 512], BF16, tag="ptr")
                    for i in range(4):
                        k = 4 * kg + i
                        nc.tensor.transpose(p[:, 128 * i:128 * (i + 1)],
                                            src2[:, 128 * k:128 * (k + 1)], ident[:])
                    dst = dstT3[:, 4 * kg:4 * (kg + 1), 128 * t:128 * (t + 1)]
                    nc.vector.tensor_copy(dst, p[:].rearrange("p (i c) -> p i c", i=4))

            with (tc.tile_pool(name="pst", bufs=2, space="PSUM") as pst,
                  tc.tile_pool(name="pp", bufs=4, space="PSUM") as pp):
                for t in range(NT):
                    transpose_tile(pst, xT3, xn3[:, t, :], t)
                for dc in range(DC):
                    p = pp.tile([128, NC], F32, tag="pp")
                    for k in range(KC):
                        nc.tensor.matmul(p[:], wq3[:, k, 128 * dc:128 * (dc + 1)],
                                         xT3[:, k, :], start=(k == 0), stop=(k == KC - 1))
                    nc.scalar.copy(qT[dc][:], p[:])

                for g in range(4):
                    cg = ctxn[g][:].rearrange("p (t k) -> p t k", t=4)
                    for t in range(4):
                        transpose_tile(pst, ctxT3, cg[:, t, :], 4 * g + t)
                    for dc in range(DC):
                        p = pp.tile([128, 512], F32, tag="pp")
                        for k in range(KC):
                            nc.tensor.matmul(p[:], wk3[:, k, 128 * dc:128 * (dc + 1)],
                                             ctxT3[:, k, 512 * g:512 * (g + 1)],
                                             start=(k == 0), stop=(k == KC - 1))
                        nc.scalar.copy(kT[dc][:, 512 * g:512 * (g + 1)], p[:])
                    for t in range(4):
                        mt = 4 * g + t
                        p = pp.tile([128, 512], F32, tag="pp")
                        for k in range(KC):
                            nc.tensor.matmul(p[:], ctxT3[:, k, 128 * mt:128 * (mt + 1)],
                                             wv3[:, k, :], start=(k == 0), stop=(k == KC - 1))
                        pv = p[:].rearrange("p (hp two d) -> p hp two d", hp=DC, two=2)
                        tv = vaug[mt][:].rearrange("p (hp w) -> p hp w", hp=DC)[
                            :, :, 0:VW].rearrange(
                            "p hp (two dd) -> p hp two dd", two=2)[:, :, :, 0:D]
                        nc.vector.tensor_copy(tv, pv)

            # ---------- attention ----------
            NG = MC // GRP  # 8 groups of 2 m-chunks
            with (tc.tile_pool(name="psS", bufs=3, space="PSUM") as psS,
                  tc.tile_pool(name="psO", bufs=2, space="PSUM") as psO,
                  tc.tile_pool(name="se", bufs=3) as se):
                for hp in range(DC):
                    for head in range(2):
                        bk = 64 * head
                        vb = VW * hp + (D + 1) * head
                        oX = psO.tile([128, NC], F32, tag="oX")

                        def do_av(pend):
                            g_p, e_p = pend
                            for i in range(GRP):
                                mc = GRP * g_p + i
                                v = vaug[mc][:, vb:vb + D + 1]
                                nc.tensor.matmul(oX[0:D + 1, :], v,
                                                 e_p[:, 512 * i:512 * (i + 1)],
                                                 start=(mc == 0), stop=(mc == MC - 1))

                        pending = None
                        for g in range(NG):
                            s = psS.tile([128, 512 * GRP], F32, tag="s")
                            for i in range(GRP):
                                mc = GRP * g + i
                                ksl = kT[hp][bk:bk + 64, 128 * mc:128 * (mc + 1)]
                                nc.tensor.matmul(s[:, 512 * i:512 * (i + 1)], ksl,
                                                 qT[hp][bk:bk + 64, :],
                                                 start=True, stop=True)
                            if pending is not None:
                                do_av(pending)
                            e = se.tile([128, 512 * GRP], BF16, tag="e")
                            nc.scalar.activation(e[:], s[:], AF.Exp, bias=0.0, scale=SCALE)
                            pending = (g, e)
                        do_av(pending)
                        den = stage.tile([1, NC], F32, tag="den")
                        nc.scalar.copy(den[:], oX[D:D + 1, :])
                        rec = stage.tile([1, NC], F32, tag="rec")
                        nc.vector.reciprocal_approx_fast(rec[:], den[:])
                        rec_b = stage.tile([D, NC], F32, tag="rec_b")
                        nc.gpsimd.partition_broadcast(rec_b[:], rec[:])
                        nc.vector.tensor_mul(oT[hp][bk:bk + D, :], oX[0:D, :], rec_b[:])

            # ---------- O projection + bias ----------
            with (tc.tile_pool(name="psD", bufs=2, space="PSUM") as psD,
                  tc.tile_pool(name="so", bufs=2) as so):
                for nt in range(NT):
                    for hf in range(2):
                        p = psD.tile([128, 512], F32, tag="pout")
                        for ic in range(DC):
                            nc.tensor.matmul(p[:], oT[ic][:, 128 * nt:128 * (nt + 1)],
                                             wo3[:, ic, 512 * hf:512 * (hf + 1)],
                                             start=(ic == 0), stop=(ic == DC - 1))
                        osb = so.tile([128, 512], F32, tag="osb")
                        nc.vector.tensor_add(osb[:], p[:], bias_bc[:, 512 * hf:512 * (hf + 1)])
                        nc.sync.dma_start(
                            OUT[128 * nt:128 * (nt + 1), 512 * hf:512 * (hf + 1)], osb[:])
    nc.compile()
    return nc


def shard_inputs(inputs):
    """full inputs dict -> list of 8 per-core in_maps"""
    x, ctx = np.asarray(inputs["x"]), np.asarray(inputs["context"])
    maps = []
    for c in range(8):
        b, j = c // 4, c % 4
        maps.append({
            "xc": np.ascontiguousarray(x[b, NC * j:NC * (j + 1), :]),
            "ctxc": np.ascontiguousarray(ctx[b]),
            "Wq": np.asarray(inputs["Wq"]), "Wk": np.asarray(inputs["Wk"]),
            "Wv": np.asarray(inputs["Wv"]), "Wo": np.asarray(inputs["Wo"]),
            "bo": np.asarray(inputs["bo"]).reshape(1, KDIM),
        })
    return maps


def unshard_outputs(results):
    out = np.empty((B, N, KDIM), dtype=np.float32)
    for c in range(8):
        b, j = c // 4, c % 4
        out[b, NC * j:NC * (j + 1), :] = results[c]["outc"]
    return out


_CACHED = {}


def kernel(**inputs):
    """Full unsharded inputs -> full output [2, 2048, 1024] fp32. Runs on 8 NeuronCores."""
    from concourse.bass_utils import run_bass_kernel_spmd
    if "nc" not in _CACHED:
        _CACHED["nc"] = build_kernel()
    nc = _CACHED["nc"]
    maps = shard_inputs(inputs)
    res = run_bass_kernel_spmd(nc, maps, list(range(8)))
    return unshard_outputs(res.results)
